# revision 11
# baseline (speedup 1.0000x reference)
"""Trainium2 Bass kernel for nn_CIRNet: 1M-step CIR-process recurrence.

Strategy
--------
Sequence-shard T=1048576 across 8 cores (L=131072 each), per-core layout
[128 partitions x 1024].  Per core:
  * one big DMA of the raw [131072,18] trace block; strided-column reads
    compute dt, sigma/epsilon projections, regs.
  * the nonlinear scan  r' = r + k(th-r)dt + sig*sqrt(|r dt|)*eps  is
    solved by defect correction: an ODE-only linear solve seeds the
    trajectory r~, then two Newton rounds each (a) evaluate the per-step
    residual rho of r~ elementwise in the reference's own operation
    order, (b) solve the linearized correction system
    delta' = A*delta + rho exactly with the hardware tensor_tensor_scan,
    (c) update r~ += delta.  The coarse ulp(1.0) rounding of A only ever
    multiplies the small delta, so no systematic bias accumulates.
  * boundary chaining across the 128 partitions uses PE transposes plus
    [1,128] scans; across the 8 cores a 2-float AllGather of the percore
    affine transfer (W_tot, Y_tot), chained locally, gives each core its
    exact incoming state per round.

Raw bass (explicit engines + semaphores): Tile's scheduler emits >2
sync-waits per instruction for this dependency shape, which this
compiler rejects.
"""

import numpy as np

import concourse.bacc as bacc
import concourse.bass as bass
import concourse.mybir as mybir

F32 = mybir.dt.float32
OP = mybir.AluOpType
ACTF = mybir.ActivationFunctionType

T = 1048576
NCORES = 8
L = T // NCORES          # 131072 sequence steps per core
P = 128
F = L // P               # 1024 per partition
N_OUT = T - 1
N_REFINE = 2

COMPUTE_ENGINES = ("act", "dve", "pool", "pe")


class Prog:
    """Two-pass emitter: collect ops with explicit deps, then emit each
    engine's stream in global order with deduped standalone sem waits.

    Compute engines get one cumulative semaphore each (in-order
    completion); every DMA and every collective gets a dedicated
    semaphore because their completions are unordered."""

    def __init__(self, nc):
        self.nc = nc
        self.ops = []  # dicts: engine, fn, deps, sem(name), amt
        self.sems = {k: nc.alloc_semaphore(f"s_{k}") for k in COMPUTE_ENGINES}
        self._next_id = 0

    def add(self, engine, fn, deps=(), collective=False):
        if engine == "sp" or collective:
            name = f"s_x{self._next_id}"
            self._next_id += 1
            self.sems[name] = self.nc.alloc_semaphore(name)
            sem, amt = name, (16 if engine == "sp" else 1)
        else:
            sem, amt = engine, 1
        self.ops.append(dict(engine=engine, fn=fn, deps=list(deps),
                             sem=sem, amt=amt))
        return len(self.ops) - 1

    def emit(self):
        nc = self.nc
        cnt = {}
        val = []
        for op in self.ops:
            cnt[op["sem"]] = cnt.get(op["sem"], 0) + op["amt"]
            val.append((op["sem"], cnt[op["sem"]]))

        def run_engine(key):
            def body(eng):
                waited = {}
                for i, op in enumerate(self.ops):
                    if op["engine"] != key:
                        continue
                    need = {}
                    for d in op["deps"]:
                        sk, sv = val[d]
                        need[sk] = max(need.get(sk, 0), sv)
                    for sk in sorted(need):
                        if need[sk] > waited.get(sk, 0):
                            eng.wait_ge(self.sems[sk], need[sk])
                            waited[sk] = need[sk]
                    instr = op["fn"](eng)
                    instr.then_inc(self.sems[op["sem"]], op["amt"])
            return body

        with nc.Block() as block:
            block.sync(run_engine("sp"))
            block.scalar(run_engine("act"))
            block.vector(run_engine("dve"))
            block.gpsimd(run_engine("pool"))
            block.tensor(run_engine("pe"))


def build(kk, th, r0, sW, sb, eW):
    """Build the SPMD program with the scalar weights baked as immediates."""
    kk = float(kk); th = float(th); r0 = float(r0); sb = float(sb)
    sW = [float(x) for x in sW]
    eW = [float(x) for x in eW]
    kth = float(np.float32(np.float32(kk) * np.float32(th)))
    reg_c = float(np.float32(np.float32(2.0) * np.float32(kk) * np.float32(th)))

    nc = bacc.Bacc("TRN2", target_bir_lowering=False, num_devices=NCORES)

    trace_d = nc.dram_tensor("traceseg", [L, 18], F32, kind="ExternalInput")
    tnext_d = nc.dram_tensor("tnext", [P, 1], F32, kind="ExternalInput")
    sel_d = nc.dram_tensor("sel", [1, 8], F32, kind="ExternalInput")
    rout_d = nc.dram_tensor("r_out", [L], F32, kind="ExternalOutput")
    regs_d = nc.dram_tensor("regs_out", [L], F32, kind="ExternalOutput")
    dts_d = nc.dram_tensor("dts_out", [L], F32, kind="ExternalOutput")
    ccin_d = [nc.dram_tensor(f"ccin{s}", [2], F32) for s in range(3)]
    ccout_d = [nc.dram_tensor(f"ccout{s}", [16], F32, addr_space="Shared")
               for s in range(3)]

    sb_ = nc.alloc_sbuf_tensor
    raw = sb_("raw", [P, F * 18], F32)
    dt = sb_("dt", [P, F], F32)
    sig = sb_("sig", [P, F], F32)
    eps = sb_("eps", [P, F], F32)
    c2 = sb_("c2", [P, F], F32)
    a_t = sb_("a_t", [P, F], F32)
    b_t = sb_("b_t", [P, F], F32)
    regs = sb_("regs", [P, F], F32)
    rt = sb_("rt", [P, F], F32)
    g = sb_("g", [P, F], F32)
    u = sb_("u", [P, F], F32)       # sqrt scratch (sqh, then u per round)
    sq = sb_("sq", [P, F], F32)
    v = sb_("v", [P, F], F32)
    A_t = sb_("A_t", [P, F], F32)
    q = sb_("q", [P, F], F32)       # se, then q per round
    w1 = sb_("w1", [P, F], F32)     # sig^2, then w1/sde per round
    p2 = sb_("p2", [P, F], F32)
    ode = sb_("ode", [P, F], F32)   # ode/step/rho in place
    W_t = sb_("W_t", [P, F], F32)
    Y_t = sb_("Y_t", [P, F], F32)
    dlt = sb_("dlt", [P, F], F32)
    zeros = sb_("zeros", [P, F], F32)
    ident = sb_("ident", [P, P], F32)
    tn = sb_("tn", [P, 1], F32)
    selt = sb_("selt", [1, 8], F32)
    zp = sb_("zp", [P, 1], F32)
    wT = sb_("wT", [1, P], F32)
    yT = sb_("yT", [1, P], F32)
    chW = sb_("chW", [1, P], F32)
    chY = sb_("chY", [1, P], F32)
    rowC = sb_("rowC", [1, P], F32)
    rowT = sb_("rowT", [1, P], F32)
    zch = sb_("zch", [1, 8], F32)
    zsh = sb_("zsh", [1, 8], F32)
    zsel = sb_("zsel", [1, 8], F32)
    zc = sb_("zc", [1, 1], F32)
    ccsb = [sb_(f"ccsb{s}", [1, 2], F32) for s in range(3)]
    agg = [sb_(f"agg{s}", [1, 16], F32) for s in range(3)]
    ps = nc.alloc_psum_tensor
    psW = ps("psW", [1, P], F32)
    psY = ps("psY", [1, P], F32)
    psZ = ps("psZ", [P, 1], F32)
    prev_reads = {"w": [], "y": [], "z": []}  # DVE reads of psum, for PE WAR

    xs = raw[:].rearrange("p (i c) -> p i c", c=18)
    pr = Prog(nc)

    # ---------------- loads & constants ----------------
    d_raw = pr.add("sp", lambda e: e.dma_start(raw[:], trace_d[:].rearrange(
        "(p q) c -> p (q c)", p=P)))
    d_tn = pr.add("sp", lambda e: e.dma_start(tn[:], tnext_d[:]))
    d_sel = pr.add("sp", lambda e: e.dma_start(selt[:], sel_d[:]))
    p_zero = pr.add("pool", lambda e: e.memset(zeros[:], 0.0))
    p_id0 = pr.add("pool", lambda e: e.memset(ident[:], 0.0))
    p_id1 = pr.add("pool", lambda e: e.affine_select(
        out=ident[:], in_=ident[:], compare_op=OP.not_equal, fill=1.0,
        base=0, pattern=[[-1, P]], channel_multiplier=1), deps=[p_id0])

    # ---------------- column extraction ----------------
    v_dt = pr.add("dve", lambda e: e.tensor_tensor(
        dt[:, 0:F - 1], xs[:, 1:F, 0], xs[:, 0:F - 1, 0], OP.subtract),
        deps=[d_raw])
    v_dtl = pr.add("dve", lambda e: e.tensor_tensor(
        dt[:, F - 1:F], tn[:], xs[:, F - 1:F, 0], OP.subtract),
        deps=[d_raw, d_tn])
    dt_ready = [v_dt, v_dtl]
    a_a = pr.add("act", lambda e: e.activation(
        a_t[:], dt[:], ACTF.Copy, bias=1.0, scale=-kk), deps=dt_ready)
    a_b = pr.add("act", lambda e: e.activation(
        b_t[:], dt[:], ACTF.Copy, bias=0.0, scale=kth), deps=dt_ready)
    a_sqh = pr.add("act", lambda e: e.activation(
        u[:], dt[:], ACTF.Sqrt, bias=0.0, scale=0.25), deps=dt_ready)

    last = pr.add("dve", lambda e: e.tensor_scalar(
        sig[:], xs[:, :, 2], sW[0], sb, OP.mult, OP.add), deps=[d_raw])
    for j in range(1, 8):
        last = pr.add("dve", lambda e, j=j: e.scalar_tensor_tensor(
            sig[:], xs[:, :, 2 + j], sW[j], sig[:], OP.mult, OP.add),
            deps=[d_raw, last])
    v_sig = last
    last = pr.add("dve", lambda e: e.tensor_scalar(
        eps[:], xs[:, :, 10], eW[0], None, OP.mult), deps=[d_raw])
    for j in range(1, 8):
        last = pr.add("dve", lambda e, j=j: e.scalar_tensor_tensor(
            eps[:], xs[:, :, 10 + j], eW[j], eps[:], OP.mult, OP.add),
            deps=[d_raw, last])
    p_eps = last

    v_se = pr.add("dve", lambda e: e.tensor_tensor(
        q[:], sig[:], eps[:], OP.mult), deps=[v_sig, p_eps])
    v_c2 = pr.add("dve", lambda e: e.tensor_tensor(
        c2[:], q[:], u[:], OP.mult), deps=[v_se, a_sqh])
    a_sq2 = pr.add("act", lambda e: e.activation(
        w1[:], sig[:], ACTF.Square, bias=0.0, scale=1.0), deps=[v_sig])
    v_regs = pr.add("dve", lambda e: e.tensor_scalar(
        regs[:], w1[:], -1.0, reg_c, OP.mult, OP.add), deps=[a_sq2])
    d_regs = pr.add("sp", lambda e: e.dma_start(
        regs_d[:].rearrange("(p f) -> p f", p=P), regs[:]), deps=[v_regs])
    d_dts = pr.add("sp", lambda e: e.dma_start(
        dts_d[:].rearrange("(p f) -> p f", p=P), dt[:]), deps=dt_ready)

    # ------------- shared linear-solve stage -------------
    SC = (OP.mult, OP.add)

    def solve_stage(s, A_ap, B_ap, z_init, out_ap, dep_A, dep_B, war_W):
        """Two-level linear solve of state' = A*state + B with global
        chaining; writes out_ap = W*zp + Y0 (the per-step states).
        Returns (final op id, zp-copy op id)."""
        scW = pr.add("dve", lambda e: e.tensor_tensor_scan(
            W_t[:], A_ap, zeros[:], 1.0, *SC),
            deps=dep_A + [p_zero] + war_W)
        scY = pr.add("dve", lambda e: e.tensor_tensor_scan(
            Y_t[:], A_ap, B_ap, 0.0, *SC), deps=dep_A + dep_B)
        tw = pr.add("pe", lambda e: e.transpose(
            psW[:], W_t[:, F - 1:F], ident[:]),
            deps=[scW, p_id1] + prev_reads["w"])
        ty = pr.add("pe", lambda e: e.transpose(
            psY[:], Y_t[:, F - 1:F], ident[:]),
            deps=[scY, p_id1] + prev_reads["y"])
        cw = pr.add("dve", lambda e: e.tensor_copy(wT[:], psW[:]),
                    deps=[tw])
        cy = pr.add("dve", lambda e: e.tensor_copy(yT[:], psY[:]),
                    deps=[ty])
        prev_reads["w"] = [cw]
        prev_reads["y"] = [cy]
        mW = pr.add("dve", lambda e: e.tensor_tensor_scan(
            chW[:], wT[:], zeros[0:1, 0:P], 1.0, *SC), deps=[cw, p_zero])
        mY = pr.add("dve", lambda e: e.tensor_tensor_scan(
            chY[:], wT[:], yT[:], 0.0, *SC), deps=[cw, cy])
        cc0 = pr.add("dve", lambda e: e.tensor_copy(
            ccsb[s][0:1, 0:1], chW[0:1, P - 1:P]), deps=[mW])
        cc1 = pr.add("dve", lambda e: e.tensor_copy(
            ccsb[s][0:1, 1:2], chY[0:1, P - 1:P]), deps=[mY])
        dcc = pr.add("sp", lambda e: e.dma_start(
            ccin_d[s][:], ccsb[s][:]), deps=[cc0, cc1])
        ag = pr.add("pool", lambda e: e.collective_compute(
            "AllGather", OP.bypass, replica_groups=[list(range(NCORES))],
            ins=[ccin_d[s][:]], outs=[ccout_d[s][:]]), deps=[dcc],
            collective=True)
        dag = pr.add("sp", lambda e: e.dma_start(
            agg[s][:], ccout_d[s][:].rearrange("(p f) -> p f", p=1)),
            deps=[ag])
        aggv = agg[s][:].rearrange("p (i c) -> p i c", c=2)
        zchain = pr.add("dve", lambda e: e.tensor_tensor_scan(
            zch[:], aggv[:, :, 0], aggv[:, :, 1], z_init, *SC), deps=[dag])
        zs1 = pr.add("dve", lambda e: e.tensor_copy(
            zsh[0:1, 1:8], zch[0:1, 0:7]), deps=[zchain])
        zs0 = pr.add("dve", lambda e: e.memset(zsh[0:1, 0:1], z_init),
                     deps=[])
        zm = pr.add("dve", lambda e: e.tensor_tensor(
            zsel[:], zsh[:], selt[:], OP.mult), deps=[zs1, zs0, d_sel])
        zr = pr.add("dve", lambda e: e.tensor_reduce(
            zc[:], zsel[:], mybir.AxisListType.X, OP.add), deps=[zm])
        row = pr.add("dve", lambda e: e.tensor_tensor_scan(
            rowC[:], wT[:], yT[:], zc[:], *SC), deps=[zr])
        rs1 = pr.add("dve", lambda e: e.tensor_copy(
            rowT[0:1, 1:P], rowC[0:1, 0:P - 1]), deps=[row])
        rs0 = pr.add("dve", lambda e: e.tensor_copy(
            rowT[0:1, 0:1], zc[:]), deps=[zr])
        tz = pr.add("pe", lambda e: e.transpose(
            psZ[:], rowT[:], ident[0:1, 0:1]),
            deps=[rs1, rs0] + prev_reads["z"])
        cz = pr.add("dve", lambda e: e.tensor_copy(zp[:], psZ[:]),
                    deps=[tz])
        prev_reads["z"] = [cz]
        fin = pr.add("dve", lambda e: e.scalar_tensor_tensor(
            out_ap, W_t[:], zp[:], Y_t[:], OP.mult, OP.add), deps=[cz])
        return fin, cz

    # ---------------- ODE seed stage ----------------
    fin0, cz0 = solve_stage(
        0, a_t[:], b_t[:], r0, rt[:], dep_A=[a_a], dep_B=[a_b], war_W=[])
    g1 = pr.add("act", lambda e: e.activation(
        g[:, 1:F], rt[:, 0:F - 1], ACTF.Copy), deps=[fin0])
    g0 = pr.add("dve", lambda e: e.tensor_copy(g[:, 0:1], zp[:]),
                deps=[cz0, fin0])
    g_ready = [g1, g0]
    rt_ready = fin0

    # ---------------- refinement rounds ----------------
    for rnd in range(N_REFINE):
        s = rnd + 1
        r_u = pr.add("act", lambda e: e.activation(
            u[:], g[:], ACTF.Sqrt, bias=0.0, scale=1.0),
            deps=g_ready + [v_c2])           # WAR: u held sqh, read by v_c2
        r_v = pr.add("dve", lambda e: e.reciprocal(v[:], u[:]), deps=[r_u])
        r_A1 = pr.add("dve", lambda e: e.tensor_tensor(
            A_t[:], c2[:], v[:], OP.mult), deps=[r_v, v_c2])
        r_A = pr.add("dve", lambda e: e.tensor_tensor(
            A_t[:], A_t[:], a_t[:], OP.add), deps=[r_A1, a_a])
        r_q = pr.add("pool", lambda e: e.tensor_tensor(
            q[:], g[:], dt[:], OP.mult), deps=g_ready + [v_c2])  # WAR: q=se
        r_sq = pr.add("act", lambda e: e.activation(
            sq[:], q[:], ACTF.Sqrt, bias=0.0, scale=1.0), deps=[r_q])
        r_w1 = pr.add("pool", lambda e: e.tensor_tensor(
            w1[:], sig[:], sq[:], OP.mult), deps=[r_sq, v_regs])  # WAR sig^2
        r_sd = pr.add("pool", lambda e: e.tensor_tensor(
            w1[:], w1[:], eps[:], OP.mult), deps=[r_w1])
        r_p2 = pr.add("dve", lambda e: e.tensor_scalar(
            p2[:], g[:], th, -kk, OP.subtract, OP.mult), deps=g_ready)
        r_p3 = pr.add("pool", lambda e: e.tensor_tensor(
            p2[:], p2[:], dt[:], OP.mult), deps=[r_p2])
        r_od = pr.add("dve", lambda e: e.tensor_tensor(
            ode[:], g[:], p2[:], OP.add), deps=[r_p3] + g_ready)
        r_st = pr.add("dve", lambda e: e.tensor_tensor(
            ode[:], ode[:], w1[:], OP.add), deps=[r_od, r_sd])
        r_rh = pr.add("dve", lambda e: e.tensor_tensor(
            ode[:], ode[:], rt[:], OP.subtract), deps=[r_st, rt_ready])

        fin, cz = solve_stage(
            s, A_t[:], ode[:], 0.0, dlt[:],
            dep_A=[r_A], dep_B=[r_rh], war_W=[])
        r_up = pr.add("dve", lambda e: e.tensor_tensor(
            rt[:], rt[:], dlt[:], OP.add), deps=[fin, g1])  # WAR: act read rt
        rt_ready = r_up
        if rnd < N_REFINE - 1:
            g1 = pr.add("act", lambda e: e.activation(
                g[:, 1:F], rt[:, 0:F - 1], ACTF.Copy),
                deps=[r_up, r_sd, r_p3])     # WAR: pool reads of g done
            g0 = pr.add("dve", lambda e: e.tensor_tensor(
                g[:, 0:1], g[:, 0:1], zp[:], OP.add), deps=[cz, r_up])
            g_ready = [g1, g0]

    pr.add("sp", lambda e: e.dma_start(
        rout_d[:].rearrange("(p f) -> p f", p=P), rt[:]), deps=[rt_ready])

    pr.emit()
    nc.compile()
    return nc


_CACHE = {}
LAST_RESULTS = None


def _get_nc(key, *args):
    if key not in _CACHE:
        _CACHE[key] = build(*args)
    return _CACHE[key]


def make_in_maps(trace, sW, sb, eW):
    trace = np.ascontiguousarray(trace, dtype=np.float32)
    in_maps = []
    for c in range(NCORES):
        seg = np.ascontiguousarray(trace[c * L:(c + 1) * L])
        tnext = np.empty((P, 1), np.float32)
        for p in range(P):
            row = min(c * L + (p + 1) * F, T - 1)
            tnext[p, 0] = trace[row, 0]
        sel = np.zeros((1, 8), np.float32)
        sel[0, c] = 1.0
        in_maps.append({"traceseg": seg, "tnext": tnext, "sel": sel})
    return in_maps


def kernel(**inputs):
    from concourse.bass_utils import run_bass_kernel_spmd

    trace = np.asarray(inputs["trace_data"], dtype=np.float32)
    sW = np.asarray(inputs["sigma_W"], np.float32)[0]
    sb = float(np.asarray(inputs["sigma_b"], np.float32)[0])
    eW = np.asarray(inputs["eps_W"], np.float32)[0]
    kk = float(np.asarray(inputs["k"], np.float32)[0])
    th = float(np.asarray(inputs["theta"], np.float32)[0])
    r0 = float(trace[0, 1])

    key = (kk, th, r0, tuple(sW.tolist()), sb, tuple(eW.tolist()))
    nc = _get_nc(key, kk, th, r0, sW, sb, eW)
    in_maps = make_in_maps(trace, sW, sb, eW)
    res = run_bass_kernel_spmd(nc, in_maps, core_ids=list(range(NCORES)))
    global LAST_RESULTS
    LAST_RESULTS = res
    r = np.concatenate([res.results[c]["r_out"] for c in range(NCORES)])[:N_OUT]
    regs = np.concatenate(
        [res.results[c]["regs_out"] for c in range(NCORES)])[:N_OUT]
    dts = np.concatenate(
        [res.results[c]["dts_out"] for c in range(NCORES)])[:N_OUT]
    return (np.ascontiguousarray(r), np.ascontiguousarray(regs),
            np.ascontiguousarray(dts))


# revision 38
# speedup vs baseline: 1.1166x; 1.1166x over previous
"""Trainium2 Bass kernel for nn_CIRNet: 1M-step CIR-process recurrence.

Strategy
--------
Sequence-shard T=1048576 across 8 cores (L=131072 each), per-core layout
[128 partitions x 1024].  Per core:
  * raw [131072,18] trace block DMA'd in 4 chunks, with strided-column
    extraction (dt, sigma/epsilon projections, regs) pipelined under the
    DMA.
  * the nonlinear scan  r' = r + k(th-r)dt + sig*sqrt(|r dt|)*eps  is
    solved by defect correction: an ODE-only linear solve seeds the
    trajectory r~, then two Newton rounds each (a) evaluate the per-step
    residual rho of r~ elementwise in (nearly) the reference's own
    operation order, (b) solve the linearized correction system
    delta' = A*delta + rho exactly with the hardware tensor_tensor_scan,
    (c) update r~ += delta.  The coarse ulp(1.0) rounding of A only ever
    multiplies the small correction delta, so no systematic bias
    accumulates the way a direct A*r+B scan would.
  * boundary chaining across the 128 partitions uses PE transposes plus
    [1,128] scans; across the 8 cores a 2-float AllGather of the percore
    affine transfer (W_tot, Y_tot), chained locally, gives each core its
    exact incoming state per round.  The homogeneous solution W is
    computed once from the ODE system and reused by the refinement
    rounds (A differs from a by ~1e-4 relatively; the resulting boundary
    error is O(1e-7) on the tiny corrections).

Raw bass (explicit engines + semaphores): Tile's scheduler emits >2
sync-waits per instruction for this dependency shape, which this
compiler rejects.
"""

import numpy as np

import concourse.bacc as bacc
import concourse.bass as bass
import concourse.mybir as mybir

F32 = mybir.dt.float32
OP = mybir.AluOpType
ACTF = mybir.ActivationFunctionType

T = 1048576
NCORES = 8
L = T // NCORES          # 131072 sequence steps per core
P = 128
F = L // P               # 1024 per partition
N_OUT = T - 1
N_REFINE = 2

COMPUTE_ENGINES = ("act", "dve", "pool", "pe")


class Prog:
    """Two-pass emitter: collect ops with explicit deps, then emit each
    engine's stream in global order with deduped standalone sem waits.

    Compute engines get one cumulative semaphore each (in-order
    completion); every DMA and every collective gets a dedicated
    semaphore because their completions are unordered."""

    def __init__(self, nc):
        self.nc = nc
        self.ops = []
        self.sems = {k: nc.alloc_semaphore(f"s_{k}") for k in COMPUTE_ENGINES}
        self._next_id = 0

    def add(self, engine, fn, deps=(), collective=False):
        if engine == "sp" or collective:
            name = f"s_x{self._next_id}"
            self._next_id += 1
            self.sems[name] = self.nc.alloc_semaphore(name)
            sem, amt = name, (16 if engine == "sp" else 1)
        else:
            sem, amt = engine, 1
        self.ops.append(dict(engine=engine, fn=fn, deps=list(deps),
                             sem=sem, amt=amt))
        return len(self.ops) - 1

    def emit(self):
        nc = self.nc
        cnt = {}
        val = []
        for op in self.ops:
            cnt[op["sem"]] = cnt.get(op["sem"], 0) + op["amt"]
            val.append((op["sem"], cnt[op["sem"]]))

        def run_engine(key):
            def body(eng):
                waited = {}
                for i, op in enumerate(self.ops):
                    if op["engine"] != key:
                        continue
                    need = {}
                    for d in op["deps"]:
                        sk, sv = val[d]
                        need[sk] = max(need.get(sk, 0), sv)
                    for sk in sorted(need):
                        if need[sk] > waited.get(sk, 0):
                            eng.wait_ge(self.sems[sk], need[sk])
                            waited[sk] = need[sk]
                    instr = op["fn"](eng)
                    instr.then_inc(self.sems[op["sem"]], op["amt"])
            return body

        with nc.Block() as block:
            block.sync(run_engine("sp"))
            block.scalar(run_engine("act"))
            block.vector(run_engine("dve"))
            block.gpsimd(run_engine("pool"))
            block.tensor(run_engine("pe"))


def build(kk, th, r0, sW, sb, eW):
    """Build the SPMD program with the scalar weights baked as immediates."""
    kk = float(kk); th = float(th); r0 = float(r0); sb = float(sb)
    sW = [float(x) for x in sW]
    eW = [float(x) for x in eW]
    kth = float(np.float32(np.float32(kk) * np.float32(th)))
    reg_c = float(np.float32(np.float32(2.0) * np.float32(kk) * np.float32(th)))

    nc = bacc.Bacc("TRN2", target_bir_lowering=False, num_devices=NCORES)

    trace_d = nc.dram_tensor("traceseg", [L, 18], F32, kind="ExternalInput")
    tnext_d = nc.dram_tensor("tnext", [P, 1], F32, kind="ExternalInput")
    sel_d = nc.dram_tensor("sel", [1, 8], F32, kind="ExternalInput")
    rout_d = nc.dram_tensor("r_out", [L], F32, kind="ExternalOutput")
    regs_d = nc.dram_tensor("regs_out", [L], F32, kind="ExternalOutput")
    dts_d = nc.dram_tensor("dts_out", [L], F32, kind="ExternalOutput")
    ccin_d = [nc.dram_tensor(f"ccin{s}", [2], F32) for s in range(4)]
    ccout_d = [nc.dram_tensor(f"ccout{s}", [16], F32, addr_space="Shared")
               for s in range(4)]

    sb_ = nc.alloc_sbuf_tensor
    raw = sb_("raw", [P, F * 18], F32)
    dt = sb_("dt", [P, F], F32)
    sig = sb_("sig", [P, F], F32)
    eps = sb_("eps", [P, F], F32)
    cF = sb_("cF", [P, F], F32)      # full c = sig*eps*sqrt(dt)
    a_t = sb_("a_t", [P, F], F32)
    b_t = sb_("b_t", [P, F], F32)    # b for the ODE solve, then sqrt(dt)
    regs = sb_("regs", [P, F], F32)
    rt = sb_("rt", [P, F], F32)
    g = sb_("g", [P, F], F32)
    u = sb_("u", [P, F], F32)
    sq = sb_("sq", [P, F], F32)
    v = sb_("v", [P, F], F32)
    A_t = sb_("A_t", [P, F], F32)
    q = sb_("q", [P, F], F32)
    w1 = sb_("w1", [P, F], F32)
    p2 = sb_("p2", [P, F], F32)
    ode = sb_("ode", [P, F], F32)
    W_t = sb_("W_t", [P, F], F32)
    Y_t = sb_("Y_t", [P, F], F32)
    dlt = sb_("dlt", [P, F], F32)
    zeros = sb_("zeros", [P, F], F32)
    ident = sb_("ident", [P, P], F32)
    tn = sb_("tn", [P, 1], F32)
    selt = sb_("selt", [1, 8], F32)
    zp = sb_("zp", [P, 1], F32)
    wT = sb_("wT", [1, P], F32)
    yT = sb_("yT", [1, P], F32)
    chW = sb_("chW", [1, P], F32)
    chY = sb_("chY", [1, P], F32)
    rowC = sb_("rowC", [1, P], F32)
    rowT = sb_("rowT", [1, P], F32)
    zch = sb_("zch", [1, 8], F32)
    zsh = sb_("zsh", [1, 8], F32)
    zsel = sb_("zsel", [1, 8], F32)
    zc = sb_("zc", [1, 1], F32)
    ccsb = [sb_(f"ccsb{s}", [1, 2], F32) for s in range(3)]
    agg = [sb_(f"agg{s}", [1, 16], F32) for s in range(3)]
    psW = nc.alloc_psum_tensor("psW", [1, P], F32)
    psY = nc.alloc_psum_tensor("psY", [1, P], F32)
    psZ = nc.alloc_psum_tensor("psZ", [P, 1], F32)

    xs = raw[:].rearrange("p (i c) -> p i c", c=18)
    pr = Prog(nc)
    SC = (OP.mult, OP.add)
    RG = [list(range(NCORES))]

    # ---------------- loads + warmup collective ----------------
    NCH = 4
    CH = F // NCH
    trv = trace_d[:].rearrange("(p q) c -> p (q c)", p=P)
    d_ch = [pr.add("sp", lambda e, j=j: e.dma_start(
        raw[:, j * CH * 18:(j + 1) * CH * 18],
        trv[:, j * CH * 18:(j + 1) * CH * 18])) for j in range(NCH)]
    # warmup collective (DRAM->DRAM seed, no compute deps): pre-initializes
    # the CC path and absorbs launch skew before the first real chain.
    d_wz = pr.add("sp", lambda e: e.dma_start(
        ccin_d[3][:], trace_d[0:1, 0:2].rearrange("a b -> (a b)")))
    pr.add("pool", lambda e: e.collective_compute(
        "AllGather", OP.bypass, replica_groups=RG,
        ins=[ccin_d[3][:]], outs=[ccout_d[3][:]]), deps=[d_wz],
        collective=True)
    d_tn = pr.add("sp", lambda e: e.dma_start(tn[:], tnext_d[:]))
    d_sel = pr.add("sp", lambda e: e.dma_start(selt[:], sel_d[:]))
    p_zero = pr.add("pool", lambda e: e.memset(zeros[:], 0.0))
    p_id0 = pr.add("pool", lambda e: e.memset(ident[:], 0.0))
    p_id1 = pr.add("pool", lambda e: e.affine_select(
        out=ident[:], in_=ident[:], compare_op=OP.not_equal, fill=1.0,
        base=0, pattern=[[-1, P]], channel_multiplier=1), deps=[p_id0])

    # ---------------- extraction (pipelined under the DMA) ----------------
    # DVE: per chunk j: sigma quarter first (gated only on chunk j), then
    # the dt chunk (which also needs chunk j+1 and would stall the stream)
    v_dtj = []
    v_sigj = []
    for j in range(NCH):
        lo, hi = j * CH, (j + 1) * CH
        last = pr.add("dve", lambda e, lo=lo, hi=hi: e.tensor_scalar(
            sig[:, lo:hi], xs[:, lo:hi, 2], sW[0], sb, OP.mult, OP.add),
            deps=[d_ch[j]])
        for jj in range(1, 8):
            last = pr.add("dve", lambda e, jj=jj, lo=lo, hi=hi:
                          e.scalar_tensor_tensor(
                sig[:, lo:hi], xs[:, lo:hi, 2 + jj], sW[jj], sig[:, lo:hi],
                OP.mult, OP.add), deps=[last])
        v_sigj.append(last)
        if j < NCH - 1:
            v_dtj.append(pr.add("dve", lambda e, lo=lo, hi=hi: e.tensor_tensor(
                dt[:, lo:hi], xs[:, lo + 1:hi + 1, 0], xs[:, lo:hi, 0],
                OP.subtract), deps=[d_ch[j], d_ch[j + 1]]))
        else:
            v_dtj.append(pr.add("dve", lambda e, lo=lo, hi=hi: e.tensor_tensor(
                dt[:, lo:hi - 1], xs[:, lo + 1:hi, 0], xs[:, lo:hi - 1, 0],
                OP.subtract), deps=[d_ch[j]]))
    v_dtl = pr.add("dve", lambda e: e.tensor_tensor(
        dt[:, F - 1:F], tn[:], xs[:, F - 1:F, 0], OP.subtract),
        deps=[d_ch[NCH - 1], d_tn])
    dt_ready = v_dtj + [v_dtl]
    v_sig = v_sigj[-1]

    # ACT stream, ordered to release the ODE scans as early as possible:
    # a/b for chunks 0-2 as their dt lands, epsilon-half0 while waiting for
    # chunk 3, then a/b chunk 3, then epsilon-half1.
    etmp = [sq, v, A_t, q, w1, p2, ode, dlt]
    H = F // 2
    a_ch, b_ch = [], []

    def emit_ab(j):
        lo, hi = j * CH, (j + 1) * CH
        dj = [v_dtj[j]] + ([v_dtl] if j == NCH - 1 else [])
        a_ch.append(pr.add("act", lambda e, lo=lo, hi=hi: e.activation(
            a_t[:, lo:hi], dt[:, lo:hi], ACTF.Copy, bias=1.0, scale=-kk),
            deps=dj))
        b_ch.append(pr.add("act", lambda e, lo=lo, hi=hi: e.activation(
            b_t[:, lo:hi], dt[:, lo:hi], ACTF.Copy, bias=0.0, scale=kth),
            deps=dj))

    eps_ops = {}  # (col, half) -> op id

    def emit_eps_half(half):
        lo, hi = (0, H) if half == 0 else (H, F)
        dmadeps = [d_ch[0], d_ch[1]] if half == 0 else [d_ch[2], d_ch[3]]
        for jj in range(8):
            eps_ops[(jj, half)] = pr.add(
                "act", lambda e, jj=jj, lo=lo, hi=hi: e.activation(
                    etmp[jj][:, lo:hi], xs[:, lo:hi, 10 + jj], ACTF.Copy,
                    bias=0.0, scale=eW[jj]), deps=dmadeps)

    for j in range(NCH - 1):
        emit_ab(j)
    emit_eps_half(0)
    emit_ab(NCH - 1)
    emit_eps_half(1)

    tree = []
    for pair in range(4):
        pops = [eps_ops[(jj, h)] for jj in (2 * pair, 2 * pair + 1)
                for h in (0, 1)]
        tree.append(pr.add("pool", lambda e, pair=pair: e.tensor_tensor(
            etmp[2 * pair][:], etmp[2 * pair][:], etmp[2 * pair + 1][:],
            OP.add), deps=pops))
    t20 = pr.add("pool", lambda e: e.tensor_tensor(
        etmp[0][:], etmp[0][:], etmp[2][:], OP.add), deps=[tree[0], tree[1]])
    t21 = pr.add("pool", lambda e: e.tensor_tensor(
        etmp[4][:], etmp[4][:], etmp[6][:], OP.add), deps=[tree[2], tree[3]])
    p_eps = pr.add("pool", lambda e: e.tensor_tensor(
        eps[:], etmp[0][:], etmp[4][:], OP.add), deps=[t20, t21])
    # v_se below waits on p_eps (pool), so every later DVE/ACT write of the
    # etmp scratch tiles is transitively ordered after the POOL tree reads.

    d_dts = pr.add("sp", lambda e: e.dma_start(
        dts_d[:].rearrange("(p f) -> p f", p=P), dt[:]), deps=dt_ready)

    # ---------------- ODE solve (also produces W/wT/chW for reuse) --------
    scW = pr.add("dve", lambda e: e.tensor_tensor_scan(
        W_t[:], a_t[:], zeros[:], 1.0, *SC), deps=a_ch + [p_zero])
    scY = pr.add("dve", lambda e: e.tensor_tensor_scan(
        Y_t[:], a_t[:], b_t[:], 0.0, *SC), deps=a_ch + b_ch)
    tw = pr.add("pe", lambda e: e.transpose(
        psW[:], W_t[:, F - 1:F], ident[:]), deps=[scW, p_id1])
    cw = pr.add("dve", lambda e: e.tensor_copy(wT[:], psW[:]), deps=[tw])
    mW = pr.add("dve", lambda e: e.tensor_tensor_scan(
        chW[:], wT[:], zeros[0:1, 0:P], 1.0, *SC), deps=[cw, p_zero])

    # filler for the stage-0 collective window: sqrt(dt) (overwrites b_t
    # once the ODE Y-scan consumed it), the c / regs products, the g
    # partial copy.
    setup_fill = {}

    def fill0():
        setup_fill["a_sqdt"] = pr.add("act", lambda e: e.activation(
            b_t[:], dt[:], ACTF.Sqrt, bias=0.0, scale=1.0),
            deps=[scY] + dt_ready)
        setup_fill["v_se"] = pr.add("dve", lambda e: e.tensor_tensor(
            cF[:], sig[:], eps[:], OP.mult), deps=[v_sig, p_eps])
        setup_fill["v_cF"] = pr.add(
            "dve", lambda e: e.scalar_tensor_tensor(
                cF[:], cF[:], 0.5, b_t[:], OP.mult, OP.mult),
            deps=[setup_fill["v_se"], setup_fill["a_sqdt"]])
        setup_fill["a_sq2"] = pr.add("act", lambda e: e.activation(
            regs[:], sig[:], ACTF.Square, bias=0.0, scale=1.0),
            deps=[v_sig])
        setup_fill["v_regs"] = pr.add("dve", lambda e: e.tensor_scalar(
            regs[:], regs[:], -1.0, reg_c, OP.mult, OP.add),
            deps=[setup_fill["a_sq2"]])
        setup_fill["g1p0"] = pr.add("act", lambda e: e.activation(
            g[:, 1:F], Y_t[:, 0:F - 1], ACTF.Copy), deps=[scY])

    prev_y_reads = []
    prev_z_reads = []
    rt_ready = None
    g_ready = []
    g1p_prev = None
    mW_cur = mW

    def boundary(s, z_init, mY_extra_dep, filler=None):
        """Y-side boundary chain for stage s (W-side from the most recent
        exact-W computation).  `filler` emits ops that execute while the
        AllGather is in flight.  Returns op id of zp copy."""
        nonlocal prev_y_reads, prev_z_reads
        cy = pr.add("dve", lambda e: e.tensor_copy(yT[:], psY[:]),
                    deps=[mY_extra_dep])
        mY = pr.add("dve", lambda e: e.tensor_tensor_scan(
            chY[:], wT[:], yT[:], 0.0, *SC), deps=[cy, mW_cur])
        cc0 = pr.add("dve", lambda e: e.tensor_copy(
            ccsb[s][0:1, 0:1], chW[0:1, P - 1:P]), deps=[mW_cur])
        cc1 = pr.add("dve", lambda e: e.tensor_copy(
            ccsb[s][0:1, 1:2], chY[0:1, P - 1:P]), deps=[mY])
        dcc = pr.add("sp", lambda e: e.dma_start(
            ccin_d[s][:], ccsb[s][:]), deps=[cc0, cc1])
        ag = pr.add("pool", lambda e: e.collective_compute(
            "AllGather", OP.bypass, replica_groups=RG,
            ins=[ccin_d[s][:]], outs=[ccout_d[s][:]]), deps=[dcc],
            collective=True)
        if filler is not None:
            filler()
        dag = pr.add("sp", lambda e: e.dma_start(
            agg[s][:], ccout_d[s][:].rearrange("(p f) -> p f", p=1)),
            deps=[ag])
        aggv = agg[s][:].rearrange("p (i c) -> p i c", c=2)
        zchain = pr.add("dve", lambda e: e.tensor_tensor_scan(
            zch[:], aggv[:, :, 0], aggv[:, :, 1], z_init, *SC), deps=[dag])
        zs1 = pr.add("dve", lambda e: e.tensor_copy(
            zsh[0:1, 1:8], zch[0:1, 0:7]), deps=[zchain])
        zs0 = pr.add("dve", lambda e: e.memset(zsh[0:1, 0:1], z_init),
                     deps=[])
        zm = pr.add("dve", lambda e: e.tensor_tensor(
            zsel[:], zsh[:], selt[:], OP.mult), deps=[zs1, zs0, d_sel])
        zr = pr.add("dve", lambda e: e.tensor_reduce(
            zc[:], zsel[:], mybir.AxisListType.X, OP.add), deps=[zm])
        row = pr.add("dve", lambda e: e.tensor_tensor_scan(
            rowC[:], wT[:], yT[:], zc[:], *SC), deps=[zr])
        rs1 = pr.add("dve", lambda e: e.tensor_copy(
            rowT[0:1, 1:P], rowC[0:1, 0:P - 1]), deps=[row])
        rs0 = pr.add("dve", lambda e: e.tensor_copy(
            rowT[0:1, 0:1], zc[:]), deps=[zr])
        tz = pr.add("pe", lambda e: e.transpose(
            psZ[:], rowT[:], ident[0:1, 0:1]),
            deps=[rs1, rs0] + prev_z_reads)
        cz = pr.add("dve", lambda e: e.tensor_copy(zp[:], psZ[:]),
                    deps=[tz])
        prev_z_reads = [cz]
        return cz

    # stage 0 (ODE)
    ty0 = pr.add("pe", lambda e: e.transpose(
        psY[:], Y_t[:, F - 1:F], ident[:]), deps=[scY, p_id1] + prev_y_reads)
    cz0 = boundary(0, r0, ty0, filler=fill0)
    prev_y_reads = [cz0]
    a_sqdt = setup_fill["a_sqdt"]
    v_cF = setup_fill["v_cF"]
    g1p0 = setup_fill["g1p0"]
    fin_rt0 = pr.add("dve", lambda e: e.scalar_tensor_tensor(
        rt[:], W_t[:], zp[:], Y_t[:], OP.mult, OP.add), deps=[cz0])
    fin_g10 = pr.add("dve", lambda e: e.scalar_tensor_tensor(
        g[:, 1:F], W_t[:, 0:F - 1], zp[:], g[:, 1:F], OP.mult, OP.add),
        deps=[cz0, g1p0])
    g00 = pr.add("dve", lambda e: e.tensor_copy(g[:, 0:1], zp[:]),
                 deps=[cz0])
    rt_ready = fin_rt0
    g_ready = [fin_g10, g00]
    g1p_prev = g1p0
    d_regs = pr.add("sp", lambda e: e.dma_start(
        regs_d[:].rearrange("(p f) -> p f", p=P), regs[:]),
        deps=[setup_fill["v_regs"]])

    # ---------------- refinement rounds ----------------
    for rnd in range(N_REFINE):
        s = rnd + 1
        lastrnd = rnd == N_REFINE - 1
        r_u = pr.add("act", lambda e: e.activation(
            u[:], g[:], ACTF.Sqrt, bias=0.0, scale=1.0), deps=g_ready)
        r_v = pr.add("dve", lambda e: e.reciprocal_approx_fast(v[:], u[:]),
                     deps=[r_u, v_cF])
        # sde = sig*sqrt(g dt)*eps = (2*cH)*u in one fused op; emitted
        # between the serially-dependent A ops so their waits pre-resolve
        r_sd = pr.add("dve", lambda e: e.scalar_tensor_tensor(
            w1[:], cF[:], 2.0, u[:], OP.mult, OP.mult),
            deps=[r_u, v_cF, p_eps])
        r_A1 = pr.add("dve", lambda e: e.tensor_tensor(
            A_t[:], cF[:], v[:], OP.mult), deps=[r_v, v_cF])
        r_A = pr.add("dve", lambda e: e.tensor_tensor(
            A_t[:], A_t[:], a_t[:], OP.add), deps=[r_A1])
        r_p2 = pr.add("act", lambda e: e.activation(
            p2[:], g[:], ACTF.Copy, bias=kth, scale=-kk),
            deps=g_ready + [p_eps])
        r_p3 = pr.add("dve", lambda e: e.tensor_tensor(
            p2[:], p2[:], dt[:], OP.mult), deps=[r_p2])
        r_od = pr.add("dve", lambda e: e.tensor_tensor(
            ode[:], g[:], p2[:], OP.add), deps=[r_p3] + g_ready)
        r_st = pr.add("dve", lambda e: e.tensor_tensor(
            ode[:], ode[:], w1[:], OP.add), deps=[r_od, r_sd])
        r_rh = pr.add("dve", lambda e: e.tensor_tensor(
            ode[:], ode[:], rt[:], OP.subtract), deps=[r_st, rt_ready])

        if rnd == 0:
            # exact W for the first Newton round (Newton-rate convergence);
            # the second round reuses it (Picard penalty only on its own
            # tiny correction).
            scWs = pr.add("dve", lambda e: e.tensor_tensor_scan(
                W_t[:], A_t[:], zeros[:], 1.0, *SC),
                deps=[r_A, fin_rt0, fin_g10])
            tws = pr.add("pe", lambda e: e.transpose(
                psW[:], W_t[:, F - 1:F], ident[:]), deps=[scWs, cw])
            cws = pr.add("dve", lambda e: e.tensor_copy(wT[:], psW[:]),
                         deps=[tws, cz0])
            mW_cur = pr.add("dve", lambda e: e.tensor_tensor_scan(
                chW[:], wT[:], zeros[0:1, 0:P], 1.0, *SC),
                deps=[cws, p_zero])

        scYs = pr.add("dve", lambda e: e.tensor_tensor_scan(
            Y_t[:], A_t[:], ode[:], 0.0, *SC),
            deps=[r_A, r_rh, g1p_prev])
        tys = pr.add("pe", lambda e: e.transpose(
            psY[:], Y_t[:, F - 1:F], ident[:]), deps=[scYs] + prev_y_reads)

        ref_fill = {}

        def fillr(lastrnd=lastrnd, scYs=scYs, rt_ready=rt_ready,
                  r_od=r_od, ref_fill=ref_fill):
            # partial update rt += Y0 overlaps the boundary chain
            ref_fill["rp"] = pr.add("dve", lambda e: e.tensor_tensor(
                rt[:], rt[:], Y_t[:], OP.add), deps=[scYs, rt_ready])
            if not lastrnd:
                ref_fill["g1ps"] = pr.add("act", lambda e: e.activation(
                    g[:, 1:F], rt[:, 0:F - 1], ACTF.Copy),
                    deps=[ref_fill["rp"], r_od])

        czs = boundary(s, 0.0, tys, filler=fillr)
        prev_y_reads = [czs]
        rp = ref_fill["rp"]
        g1ps = ref_fill.get("g1ps")
        up_rt = pr.add("dve", lambda e: e.scalar_tensor_tensor(
            rt[:], W_t[:], zp[:], rt[:], OP.mult, OP.add),
            deps=[czs, rp] + ([g1ps] if g1ps is not None else []))
        rt_ready = up_rt
        if not lastrnd:
            up_g1 = pr.add("dve", lambda e: e.scalar_tensor_tensor(
                g[:, 1:F], W_t[:, 0:F - 1], zp[:], g[:, 1:F],
                OP.mult, OP.add), deps=[czs, g1ps])
            g0s = pr.add("dve", lambda e: e.tensor_tensor(
                g[:, 0:1], g[:, 0:1], zp[:], OP.add), deps=[czs])
            g_ready = [up_g1, g0s]
            g1p_prev = g1ps

    pr.add("sp", lambda e: e.dma_start(
        rout_d[:].rearrange("(p f) -> p f", p=P), rt[:]), deps=[rt_ready])

    pr.emit()
    nc.compile()
    return nc


_CACHE = {}
LAST_RESULTS = None


def _get_nc(key, *args):
    if key not in _CACHE:
        _CACHE[key] = build(*args)
    return _CACHE[key]


def make_in_maps(trace, sW, sb, eW):
    trace = np.ascontiguousarray(trace, dtype=np.float32)
    in_maps = []
    for c in range(NCORES):
        seg = np.ascontiguousarray(trace[c * L:(c + 1) * L])
        tnext = np.empty((P, 1), np.float32)
        for p in range(P):
            row = min(c * L + (p + 1) * F, T - 1)
            tnext[p, 0] = trace[row, 0]
        sel = np.zeros((1, 8), np.float32)
        sel[0, c] = 1.0
        in_maps.append({"traceseg": seg, "tnext": tnext, "sel": sel})
    return in_maps


def kernel(**inputs):
    from concourse.bass_utils import run_bass_kernel_spmd

    trace = np.asarray(inputs["trace_data"], dtype=np.float32)
    sW = np.asarray(inputs["sigma_W"], np.float32)[0]
    sb = float(np.asarray(inputs["sigma_b"], np.float32)[0])
    eW = np.asarray(inputs["eps_W"], np.float32)[0]
    kk = float(np.asarray(inputs["k"], np.float32)[0])
    th = float(np.asarray(inputs["theta"], np.float32)[0])
    r0 = float(trace[0, 1])

    key = (kk, th, r0, tuple(sW.tolist()), sb, tuple(eW.tolist()))
    nc = _get_nc(key, kk, th, r0, sW, sb, eW)
    in_maps = make_in_maps(trace, sW, sb, eW)
    res = run_bass_kernel_spmd(nc, in_maps, core_ids=list(range(NCORES)))
    global LAST_RESULTS
    LAST_RESULTS = res
    r = np.concatenate([res.results[c]["r_out"] for c in range(NCORES)])[:N_OUT]
    regs = np.concatenate(
        [res.results[c]["regs_out"] for c in range(NCORES)])[:N_OUT]
    dts = np.concatenate(
        [res.results[c]["dts_out"] for c in range(NCORES)])[:N_OUT]
    return (np.ascontiguousarray(r), np.ascontiguousarray(regs),
            np.ascontiguousarray(dts))
